# revision 1
# baseline (speedup 1.0000x reference)
"""Causal multi-head self-attention with RoPE on 8 NeuronCores.

Sharding: batch (4) x head-halves (2) -> 8 cores. Core c handles batch c//2,
heads [8*(c%2), 8*(c%2)+8). QKV/attention are computed per-core in transposed
layout (dims on partitions, sequence on free dim); the output projection is
row-sharded over Wo with a pair AllReduce producing the full output on the
even core of each pair.
"""

import numpy as np
import ml_dtypes

import concourse.bacc as bacc
import concourse.bass as bass
import concourse.mybir as mybir
from concourse.tile import TileContext
from concourse.bass_utils import run_bass_kernel_spmd

B, S, D, H = 4, 2048, 1024, 16
HL = 8          # heads per core
DK = 64         # head dim
NCORES = 8
DT = D // 128   # 8 d-tiles (contraction tiles)
OT = HL * DK // 128   # 4 o-tiles for Q^T/K^T ([128, S] each, 2 heads per tile)
ST = S // 128   # 16 s-tiles
NCH = S // 512  # 4 sequence chunks of 512
VW = DK + 1     # V columns per head incl. ones column

BF16 = mybir.dt.bfloat16
F32 = mybir.dt.float32
NEG = -1.0e9

_compiled = {}


def _build_nc():
    nc = bacc.Bacc("TRN2", target_bir_lowering=False, debug=False,
                   num_devices=NCORES)

    xT = nc.dram_tensor("xT", [D, S], BF16, kind="ExternalInput")
    wqT = nc.dram_tensor("wqT", [D, HL * DK], BF16, kind="ExternalInput")
    wkT = nc.dram_tensor("wkT", [D, HL * DK], BF16, kind="ExternalInput")
    wvT = nc.dram_tensor("wvT", [D, HL * DK], BF16, kind="ExternalInput")
    woT = nc.dram_tensor("woT", [HL * DK, D], BF16, kind="ExternalInput")
    cosT = nc.dram_tensor("cosT", [128, S], BF16, kind="ExternalInput")
    sinT = nc.dram_tensor("sinT", [128, S], BF16, kind="ExternalInput")
    swapT = nc.dram_tensor("swapT", [128, 128], BF16, kind="ExternalInput")
    maskT = nc.dram_tensor("maskT", [128, 128], F32, kind="ExternalInput")
    y = nc.dram_tensor("y", [S, D], BF16, kind="ExternalOutput")

    groups = [[0, 1], [2, 3], [4, 5], [6, 7]]

    with TileContext(nc) as tc:
        with (
            tc.tile_pool(name="big", bufs=1) as big,
            tc.tile_pool(name="work", bufs=2) as work,
            tc.tile_pool(name="ptile", bufs=10) as ptile,
            tc.tile_pool(name="norm", bufs=2) as normp,
            tc.tile_pool(name="ps_b", bufs=3, space="PSUM") as ps_b,
            tc.tile_pool(name="ps_o", bufs=2, space="PSUM") as ps_o,
            tc.tile_pool(name="dram", bufs=1, space="DRAM") as dram,
        ):
            # ---- load constant-ish inputs ----
            w_sb = {"q": [], "k": [], "v": []}
            for name, drt in (("q", wqT), ("k", wkT), ("v", wvT)):
                for k in range(DT):
                    t = big.tile([128, HL * DK], BF16, tag=f"w{name}{k}", name=f"w{name}{k}")
                    nc.scalar.dma_start(out=t[:], in_=drt[128 * k:128 * (k + 1), :])
                    w_sb[name].append(t)
            xT_sb = []
            for k in range(DT):
                t = big.tile([128, S], BF16, tag=f"xT{k}", name=f"xT{k}")
                eng = nc.sync if k % 2 == 0 else nc.scalar
                eng.dma_start(out=t[:], in_=xT[128 * k:128 * (k + 1), :])
                xT_sb.append(t)
            woT_sb = []
            for k in range(OT):
                t = big.tile([128, D], BF16, tag=f"wo{k}", name=f"wo{k}")
                nc.sync.dma_start(out=t[:], in_=woT[128 * k:128 * (k + 1), :])
                woT_sb.append(t)
            cos_sb = big.tile([128, S], BF16, tag="cos")
            nc.scalar.dma_start(out=cos_sb[:], in_=cosT[:])
            sin_sb = big.tile([128, S], BF16, tag="sin")
            nc.scalar.dma_start(out=sin_sb[:], in_=sinT[:])
            swap_sb = big.tile([128, 128], BF16, tag="swap")
            nc.scalar.dma_start(out=swap_sb[:], in_=swapT[:])
            mask_sb = big.tile([128, 128], F32, tag="mask")
            nc.scalar.dma_start(out=mask_sb[:], in_=maskT[:])

            # ---- phase A: QKV projections + RoPE ----
            qrot_sb = [big.tile([128, S], BF16, tag=f"qr{t}", name=f"qr{t}") for t in range(OT)]
            krot_sb = [big.tile([128, S], BF16, tag=f"kr{t}", name=f"kr{t}") for t in range(OT)]
            vaug_sb = [big.tile([128, HL * VW], BF16, tag=f"va{st}", name=f"va{st}")
                       for st in range(ST)]

            for wname, rot_tiles in (("q", qrot_sb), ("k", krot_sb)):
                for t in range(OT):
                    raw = work.tile([128, S], BF16, tag="raw")
                    for n in range(NCH):
                        ps = ps_b.tile([128, 1024], F32, tag="psb", name="ps")[:, 0:512]
                        for k in range(DT):
                            nc.tensor.matmul(
                                ps[:],
                                lhsT=w_sb[wname][k][:, 128 * t:128 * (t + 1)],
                                rhs=xT_sb[k][:, 512 * n:512 * (n + 1)],
                                start=(k == 0), stop=(k == DT - 1),
                            )
                        nc.scalar.copy(out=raw[:, 512 * n:512 * (n + 1)], in_=ps[:])
                    t1 = work.tile([128, S], BF16, tag="t1", bufs=1)
                    nc.vector.tensor_mul(t1[:], raw[:], cos_sb[:])
                    t2 = work.tile([128, S], BF16, tag="t2", bufs=1)
                    for n in range(NCH):
                        sl = slice(512 * n, 512 * (n + 1))
                        ps2 = ps_b.tile([128, 1024], F32, tag="psb", name="ps2")[:, 0:512]
                        nc.tensor.matmul(ps2[:], lhsT=swap_sb[:], rhs=raw[:, sl],
                                         start=True, stop=True)
                        nc.vector.tensor_mul(t2[:, sl], ps2[:], sin_sb[:, sl])
                    nc.vector.tensor_add(rot_tiles[t][:], t1[:], t2[:])

            for st in range(ST):
                ps = ps_b.tile([128, 1024], F32, tag="psb", name="ps")[:, 0:512]
                for k in range(DT):
                    nc.tensor.matmul(
                        ps[:],
                        lhsT=xT_sb[k][:, 128 * st:128 * (st + 1)],
                        rhs=w_sb["v"][k][:],
                        start=(k == 0), stop=(k == DT - 1),
                    )
                # strided copy: head h cols [64h:64h+64] -> vaug cols [65h:65h+64]
                dst = vaug_sb[st][:].rearrange("p (h d) -> p h d", d=VW)[:, :, 0:DK]
                src = ps[:].rearrange("p (h d) -> p h d", d=DK)
                nc.vector.tensor_copy(dst, src)
                ones = vaug_sb[st][:].rearrange("p (h d) -> p h d", d=VW)[:, :, DK:VW]
                nc.gpsimd.memset(ones, 1.0)

            # ---- phase B/C: attention + projection + allreduce, per i-chunk ----
            ypart = dram.tile([S, D], BF16)
            yred = dram.tile([S, D], BF16)
            oT_sb = [big.tile([128, S], BF16, tag=f"oT{t}", name=f"oT{t}") for t in range(OT)]

            def attn_chunk(m):
                i0 = 512 * m
                njb = 4 * m + 4
                sums8 = normp.tile([HL, 512], F32, tag="sums8", bufs=1,
                                   name="sums8")
                o_sb = [None] * HL
                for tp in range(OT):  # head pair = o-tile tp (heads 2tp, 2tp+1)
                    o_pse = ps_o.tile([VW, 512], F32, tag="pso")
                    o_pso = ps_o.tile([VW, 512], F32, tag="pso")
                    pTs = [None] * njb
                    fulls = [jb for jb in range(njb) if 128 * jb < i0]
                    diags = [jb for jb in range(njb) if 128 * jb >= i0]
                    order = []
                    for idx in range(max(len(fulls), len(diags))):
                        if idx < len(diags):
                            order.append(diags[idx])
                        if idx < len(fulls):
                            order.extend(fulls[idx::4][:1])
                    # fallback: ensure every jb present exactly once
                    seen = set()
                    order = [jb for jb in order + list(range(njb))
                             if not (jb in seen or seen.add(jb))]
                    for jb in order:
                        j0 = 128 * jb
                        dlt = max(0, j0 - i0)
                        s_ps = ps_b.tile([128, 1024], F32, tag="psb")
                        for half, po in ((0, 0), (1, DK)):
                            nc.tensor.matmul(
                                s_ps[:, 512 * half + dlt:512 * (half + 1)],
                                lhsT=krot_sb[tp][po:po + DK, j0:j0 + 128],
                                rhs=qrot_sb[tp][po:po + DK, i0 + dlt:i0 + 512],
                                start=True, stop=True,
                            )
                        if j0 >= i0:
                            s3 = s_ps[:].rearrange("p (b f) -> p b f", b=2)
                            nc.vector.tensor_add(
                                s3[:, :, dlt:dlt + 128],
                                s3[:, :, dlt:dlt + 128],
                                mask_sb[:].rearrange("p (b f) -> p b f", b=1)
                                .broadcast_to([128, 2, 128]))
                        pT = ptile.tile([128, 1024], BF16, tag="pT")
                        nc.scalar.activation(
                            pT[:].rearrange("p (b f) -> p b f", b=2)[:, :, dlt:512],
                            s_ps[:].rearrange("p (b f) -> p b f", b=2)[:, :, dlt:512],
                            mybir.ActivationFunctionType.Exp, scale=0.125)
                        pTs[jb] = pT
                    for jb in range(njb):
                        dlt = max(0, 128 * jb - i0)
                        for half, o_ps in ((0, o_pse), (1, o_pso)):
                            nc.tensor.matmul(
                                o_ps[:, dlt:512],
                                lhsT=vaug_sb[jb][:, VW * (2 * tp + half):
                                                 VW * (2 * tp + half + 1)],
                                rhs=pTs[jb][:, 512 * half + dlt:512 * (half + 1)],
                                start=(jb == 0), stop=(jb == njb - 1),
                            )
                    for half, o_ps in ((0, o_pse), (1, o_pso)):
                        h = 2 * tp + half
                        osb = normp.tile([VW, 512], F32, tag=f"osb{h}", bufs=1,
                                         name=f"osb{h}")
                        nc.scalar.copy(out=osb[:], in_=o_ps[:])
                        nc.sync.dma_start(out=sums8[h:h + 1, :],
                                          in_=osb[DK:VW, :])
                        o_sb[h] = osb
                rec8 = normp.tile([HL, 512], F32, tag="rec8", bufs=1,
                                  name="rec8")
                nc.vector.reciprocal(rec8[:], sums8[:])
                for h in range(HL):
                    tp2, po = h // 2, DK * (h % 2)
                    stage = normp.tile([1, 512], F32, tag="stage", bufs=2,
                                       name="stage")
                    nc.sync.dma_start(out=stage[:], in_=rec8[h:h + 1, :])
                    rep = normp.tile([DK, 512], F32, tag="rep", bufs=2,
                                     name="rep")
                    nc.gpsimd.partition_broadcast(rep[:], stage[:])
                    nc.vector.tensor_mul(
                        oT_sb[tp2][po:po + DK, i0:i0 + 512],
                        o_sb[h][0:DK, :], rep[:])

            def proj_chunk(m, ncc):
                # projection of chunk m's rows (partial over my 512 dims)
                i0 = 512 * m
                for r2 in range(4):
                    r0 = i0 + 128 * r2
                    ych = work.tile([128, D], BF16, tag="ych")
                    for nn in range(2):
                        yp = ps_b.tile([128, 1024], F32, tag="psb", name="yp")[:, 0:512]
                        for k in range(OT):
                            nc.tensor.matmul(
                                yp[:],
                                lhsT=oT_sb[k][:, r0:r0 + 128],
                                rhs=woT_sb[k][:, 512 * nn:512 * (nn + 1)],
                                start=(k == 0), stop=(k == OT - 1),
                            )
                        nc.scalar.copy(out=ych[:, 512 * nn:512 * (nn + 1)], in_=yp[:])
                    nc.sync.dma_start(out=ypart[r0:r0 + 128, :], in_=ych[:])
                rows = 512 // ncc
                for q in range(ncc):
                    q0 = i0 + rows * q
                    nc.gpsimd.collective_compute(
                        "AllReduce", mybir.AluOpType.add, replica_groups=groups,
                        ins=[ypart[q0:q0 + rows, :].opt()],
                        outs=[yred[q0:q0 + rows, :].opt()],
                    )
                    nc.sync.dma_start(out=y[q0:q0 + rows, :],
                                      in_=yred[q0:q0 + rows, :])

            order_m = [3, 2, 1, 0]
            for idx, m in enumerate(order_m):
                attn_chunk(m)
                if idx >= 1:
                    proj_chunk(order_m[idx - 1], 1)
            proj_chunk(order_m[-1], 2)

    nc.compile()
    return nc


def _prep_inputs(x, Wq, Wk, Wv, Wo, cos_emb, sin_emb, token_positions):
    bf = ml_dtypes.bfloat16
    cos_g = np.asarray(cos_emb)[np.asarray(token_positions)]  # [S, DK]
    sin_g = np.asarray(sin_emb)[np.asarray(token_positions)]
    # [128, S]: partition p -> head-dim p % 64
    cosT = np.ascontiguousarray(np.tile(cos_g.T, (2, 1))).astype(bf)
    sinT = np.ascontiguousarray(np.tile(sin_g.T, (2, 1))).astype(bf)
    # rotate-half-interleaved as a matmul: rh = SWAP @ q (per 128-dim tile)
    swap = np.zeros((128, 128), np.float32)
    for j in range(64):
        swap[2 * j, 2 * j + 1] = -1.0
        swap[2 * j + 1, 2 * j] = 1.0
    swapT = np.ascontiguousarray(swap.T).astype(bf)
    # causal mask for the diagonal 128x128 block in S^T=[j,i] layout
    jj = np.arange(128)[:, None]
    ii = np.arange(128)[None, :]
    maskT = np.where(ii >= jj, 0.0, NEG).astype(np.float32)

    in_maps = []
    for c in range(NCORES):
        b, hh = c // 2, c % 2
        cols = slice(512 * hh, 512 * (hh + 1))
        in_maps.append({
            "xT": np.ascontiguousarray(np.asarray(x)[b].T).astype(bf),
            "wqT": np.ascontiguousarray(np.asarray(Wq)[cols, :].T).astype(bf),
            "wkT": np.ascontiguousarray(np.asarray(Wk)[cols, :].T).astype(bf),
            "wvT": np.ascontiguousarray(np.asarray(Wv)[cols, :].T).astype(bf),
            "woT": np.ascontiguousarray(np.asarray(Wo)[:, cols].T).astype(bf),
            "cosT": cosT, "sinT": sinT, "swapT": swapT, "maskT": maskT,
        })
    return in_maps


def kernel(x, Wq, Wk, Wv, Wo, cos_emb, sin_emb, token_positions, **run_kwargs):
    if "nc" not in _compiled:
        _compiled["nc"] = _build_nc()
    nc = _compiled["nc"]
    in_maps = _prep_inputs(x, Wq, Wk, Wv, Wo, cos_emb, sin_emb, token_positions)
    res = run_bass_kernel_spmd(nc, in_maps, list(range(NCORES)), **run_kwargs)
    out = np.stack([res.results[2 * b]["y"] for b in range(B)]).astype(np.float32)
    if run_kwargs:
        kernel.last_result = res
    return out



# revision 3
# speedup vs baseline: 1.1237x; 1.1237x over previous
"""Causal multi-head self-attention with RoPE on 8 NeuronCores.

Sharding: batch (4) x head-halves (2) -> 8 cores. Core c handles batch c//2,
heads [8*(c%2), 8*(c%2)+8). Pipelined schedule: QKV projections are computed
in two sequence halves, attention i-chunks and output-projection pieces are
interleaved so the Scalar engine (softmax exp) and PE (matmuls) overlap across
the whole kernel. Wo is row-sharded with a pair AllReduce per 512-row chunk.
"""

import numpy as np
import ml_dtypes

import concourse.bacc as bacc
import concourse.bass as bass
import concourse.mybir as mybir
from concourse.tile import TileContext
from concourse.bass_utils import run_bass_kernel_spmd

B, S, D, H = 4, 2048, 1024, 16
HL = 8          # heads per core
DK = 64         # head dim
NCORES = 8
DT = D // 128   # 8 d-tiles (contraction tiles)
OT = HL * DK // 128   # 4 o-tiles for Q^T/K^T ([128, S] each, 2 heads per tile)
ST = S // 128   # 16 s-tiles
VW = DK + 1     # V columns per head incl. ones column

BF16 = mybir.dt.bfloat16
F32 = mybir.dt.float32
NEG = -1.0e9
EXP = mybir.ActivationFunctionType.Exp

_compiled = {}


def _build_nc():
    nc = bacc.Bacc("TRN2", target_bir_lowering=False, debug=False,
                   num_devices=NCORES)

    xT = nc.dram_tensor("xT", [D, S], BF16, kind="ExternalInput")
    wqT = nc.dram_tensor("wqT", [D, HL * DK], BF16, kind="ExternalInput")
    wkT = nc.dram_tensor("wkT", [D, HL * DK], BF16, kind="ExternalInput")
    wvT = nc.dram_tensor("wvT", [D, HL * DK], BF16, kind="ExternalInput")
    woT = nc.dram_tensor("woT", [HL * DK, D], BF16, kind="ExternalInput")
    cosT = nc.dram_tensor("cosT", [128, S], BF16, kind="ExternalInput")
    sinT = nc.dram_tensor("sinT", [128, S], BF16, kind="ExternalInput")
    swapT = nc.dram_tensor("swapT", [128, 128], BF16, kind="ExternalInput")
    maskT = nc.dram_tensor("maskT", [128, 128], F32, kind="ExternalInput")
    y = nc.dram_tensor("y", [S, D], BF16, kind="ExternalOutput")

    groups = [[0, 1], [2, 3], [4, 5], [6, 7]]

    with TileContext(nc) as tc:
        with (
            tc.tile_pool(name="big", bufs=1) as big,
            tc.tile_pool(name="rawp", bufs=2) as rawp,
            tc.tile_pool(name="t1p", bufs=2) as t1p,
            tc.tile_pool(name="t2p", bufs=2) as t2p,
            tc.tile_pool(name="ptp", bufs=10) as ptp,
            tc.tile_pool(name="osbp", bufs=10) as osbp,
            tc.tile_pool(name="otp", bufs=8) as otp,
            tc.tile_pool(name="ychp", bufs=2) as ychp,
            tc.tile_pool(name="nrmp", bufs=2) as nrmp,
            tc.tile_pool(name="stgp", bufs=3) as stgp,
            tc.tile_pool(name="repp", bufs=3) as repp,
            tc.tile_pool(name="ps_b", bufs=3, space="PSUM") as ps_b,
            tc.tile_pool(name="ps_o", bufs=2, space="PSUM") as ps_o,
            tc.tile_pool(name="dram", bufs=1, space="DRAM") as dram,
        ):
            # ---- input DMAs, k-interleaved so QKV can start early ----
            w_sb = {"q": [None] * DT, "k": [None] * DT, "v": [None] * DT}
            xT_sb = [None] * DT
            for k in range(DT):
                wt = big.tile([128, HL * DK], BF16, tag=f"wq{k}", name=f"wq{k}")
                nc.scalar.dma_start(out=wt[:], in_=wqT[128 * k:128 * (k + 1), :])
                w_sb["q"][k] = wt
                xt = big.tile([128, S], BF16, tag=f"xT{k}", name=f"xT{k}")
                nc.sync.dma_start(out=xt[:], in_=xT[128 * k:128 * (k + 1), :])
                xT_sb[k] = xt
            cos_sb = big.tile([128, S], BF16, tag="cos")
            nc.scalar.dma_start(out=cos_sb[:], in_=cosT[:])
            sin_sb = big.tile([128, S], BF16, tag="sin")
            nc.scalar.dma_start(out=sin_sb[:], in_=sinT[:])
            swap_sb = big.tile([128, 128], BF16, tag="swap")
            nc.scalar.dma_start(out=swap_sb[:], in_=swapT[:])
            for k in range(DT):
                wt = big.tile([128, HL * DK], BF16, tag=f"wk{k}", name=f"wk{k}")
                nc.sync.dma_start(out=wt[:], in_=wkT[128 * k:128 * (k + 1), :])
                w_sb["k"][k] = wt
            for k in range(DT):
                wt = big.tile([128, HL * DK], BF16, tag=f"wv{k}", name=f"wv{k}")
                nc.sync.dma_start(out=wt[:], in_=wvT[128 * k:128 * (k + 1), :])
                w_sb["v"][k] = wt
            mask_sb = big.tile([128, 128], F32, tag="mask")
            nc.scalar.dma_start(out=mask_sb[:], in_=maskT[:])
            woT_sb = []
            for k in range(OT):
                wt = big.tile([128, D], BF16, tag=f"wo{k}", name=f"wo{k}")
                nc.sync.dma_start(out=wt[:], in_=woT[128 * k:128 * (k + 1), :])
                woT_sb.append(wt)

            vaug = []
            for st in range(ST):
                vt = big.tile([128, HL * VW], BF16, tag=f"va{st}", name=f"va{st}")
                ones = vt[:].rearrange("p (h d) -> p h d", d=VW)[:, :, DK:VW]
                nc.gpsimd.memset(ones, 1.0)
                vaug.append(vt)
            qrot = [big.tile([128, S], BF16, tag=f"qr{t}", name=f"qr{t}")
                    for t in range(OT)]
            krot = [big.tile([128, S], BF16, tag=f"kr{t}", name=f"kr{t}")
                    for t in range(OT)]

            oT = {}        # m -> [OT tiles of [128, 512]]
            osb_map = {}   # (m, h) -> osb tile
            sums_map = {}  # m -> sums8 tile
            ypart = dram.tile([S, D], BF16)
            yred = dram.tile([S, D], BF16)

            def qk_unit(h, qk, t):
                c0 = 1024 * h
                ps = ps_b.tile([128, 1024], F32, tag="psb", name="ps")
                for k in range(DT):
                    for cc in range(2):
                        nc.tensor.matmul(
                            ps[:, 512 * cc:512 * (cc + 1)],
                            lhsT=w_sb[qk][k][:, 128 * t:128 * (t + 1)],
                            rhs=xT_sb[k][:, c0 + 512 * cc:c0 + 512 * (cc + 1)],
                            start=(k == 0), stop=(k == DT - 1))
                raw = rawp.tile([128, 1024], BF16, tag="raw")
                nc.scalar.copy(out=raw[:], in_=ps[:])
                t1 = t1p.tile([128, 1024], BF16, tag="t1")
                nc.vector.tensor_mul(t1[:], raw[:], cos_sb[:, c0:c0 + 1024])
                ps2 = ps_b.tile([128, 1024], F32, tag="psb", name="ps2")
                for cc in range(2):
                    nc.tensor.matmul(
                        ps2[:, 512 * cc:512 * (cc + 1)], lhsT=swap_sb[:],
                        rhs=raw[:, 512 * cc:512 * (cc + 1)],
                        start=True, stop=True)
                t2 = t2p.tile([128, 1024], BF16, tag="t2")
                nc.vector.tensor_mul(t2[:], ps2[:], sin_sb[:, c0:c0 + 1024])
                dst = qrot[t] if qk == "q" else krot[t]
                nc.vector.tensor_add(dst[:, c0:c0 + 1024], t1[:], t2[:])

            def v_unit(st):
                ps = ps_b.tile([128, 1024], F32, tag="psb", name="psv")[:, 0:512]
                for k in range(DT):
                    nc.tensor.matmul(
                        ps[:], lhsT=xT_sb[k][:, 128 * st:128 * (st + 1)],
                        rhs=w_sb["v"][k][:], start=(k == 0), stop=(k == DT - 1))
                dst = vaug[st][:].rearrange("p (h d) -> p h d", d=VW)[:, :, 0:DK]
                src = ps[:].rearrange("p (h d) -> p h d", d=DK)
                nc.vector.tensor_copy(dst, src)

            def attn_tp(m, tp):
                i0 = 512 * m
                njb = 4 * m + 4
                diags = list(range(4 * m, njb))
                fulls = list(range(0, 4 * m))
                order = diags + fulls
                ogrps = [order[i:i + 8] for i in range(0, len(order), 8)]
                o_pse = ps_o.tile([VW, 512], F32, tag="pso", name="opse")
                o_pso = ps_o.tile([VW, 512], F32, tag="pso", name="opso")
                started = False
                ndone = 0
                for grp in ogrps:
                    pts = {}
                    for jb in grp:
                        j0 = 128 * jb
                        dlt = max(0, j0 - i0)
                        s_ps = ps_b.tile([128, 1024], F32, tag="psb", name="sps")
                        for half, po in ((0, 0), (1, DK)):
                            nc.tensor.matmul(
                                s_ps[:, 512 * half + dlt:512 * (half + 1)],
                                lhsT=krot[tp][po:po + DK, j0:j0 + 128],
                                rhs=qrot[tp][po:po + DK, i0 + dlt:i0 + 512],
                                start=True, stop=True)
                        if j0 >= i0:
                            s3 = s_ps[:].rearrange("p (b f) -> p b f", b=2)
                            nc.vector.tensor_add(
                                s3[:, :, dlt:dlt + 128],
                                s3[:, :, dlt:dlt + 128],
                                mask_sb[:].rearrange("p (b f) -> p b f", b=1)
                                .broadcast_to([128, 2, 128]))
                        pT = ptp.tile([128, 1024], BF16, tag="pT")
                        nc.scalar.activation(
                            pT[:].rearrange("p (b f) -> p b f", b=2)[:, :, dlt:512],
                            s_ps[:].rearrange("p (b f) -> p b f", b=2)[:, :, dlt:512],
                            EXP, scale=0.125)
                        pts[jb] = pT
                    ndone += len(grp)
                    for jb in sorted(grp):
                        dlt = max(0, 128 * jb - i0)
                        last = (ndone == njb and jb == max(grp))
                        for half, o_ps in ((0, o_pse), (1, o_pso)):
                            nc.tensor.matmul(
                                o_ps[:, dlt:512],
                                lhsT=vaug[jb][:, VW * (2 * tp + half):
                                              VW * (2 * tp + half + 1)],
                                rhs=pts[jb][:, 512 * half + dlt:512 * (half + 1)],
                                start=(not started), stop=last,
                            )
                        started = True
                osb_e = osbp.tile([VW, 512], BF16, tag="osb", name="osbe")
                nc.vector.tensor_copy(osb_e[:], o_pse[:])
                osb_o = osbp.tile([VW, 512], BF16, tag="osb", name="osbo")
                nc.vector.tensor_copy(osb_o[:], o_pso[:])
                if m not in sums_map:
                    sums_map[m] = nrmp.tile([8, 512], BF16, tag="sums8",
                                            name=f"sums8_{m}")
                sums8 = sums_map[m]
                nc.sync.dma_start(out=sums8[2 * tp:2 * tp + 1, :],
                                  in_=osb_e[DK:VW, :])
                nc.sync.dma_start(out=sums8[2 * tp + 1:2 * tp + 2, :],
                                  in_=osb_o[DK:VW, :])
                osb_map[(m, 2 * tp)] = osb_e
                osb_map[(m, 2 * tp + 1)] = osb_o

            def attn_finish(m):
                sums8 = sums_map[m]
                rec8 = nrmp.tile([8, 512], BF16, tag="rec8", name=f"rec8_{m}")
                with nc.allow_low_precision(reason="bf16 softmax denom ok at 2e-2"):
                    nc.vector.reciprocal(rec8[:], sums8[:])
                oT4 = [otp.tile([128, 512], BF16, tag="oT", name=f"oT{m}_{t}")
                       for t in range(OT)]
                for h in range(HL):
                    tp2, po = h // 2, DK * (h % 2)
                    stage = stgp.tile([1, 512], BF16, tag="stage")
                    nc.sync.dma_start(out=stage[:], in_=rec8[h:h + 1, :])
                    rep = repp.tile([64, 512], BF16, tag="rep")
                    nc.gpsimd.partition_broadcast(rep[:], stage[:])
                    nc.vector.tensor_mul(oT4[tp2][po:po + DK, :],
                                         osb_map[(m, h)][0:DK, :], rep[:])
                oT[m] = oT4

            def proj_piece(m, r2):
                r0 = 512 * m + 128 * r2
                ych = ychp.tile([128, 1024], BF16, tag="ych")
                for nn2 in range(2):
                    yp = ps_b.tile([128, 1024], F32, tag="psb", name="yp")[:, 0:512]
                    for kt in range(OT):
                        nc.tensor.matmul(
                            yp[:],
                            lhsT=oT[m][kt][:, 128 * r2:128 * (r2 + 1)],
                            rhs=woT_sb[kt][:, 512 * nn2:512 * (nn2 + 1)],
                            start=(kt == 0), stop=(kt == OT - 1))
                    nc.vector.tensor_copy(ych[:, 512 * nn2:512 * (nn2 + 1)], yp[:])
                nc.sync.dma_start(out=ypart[r0:r0 + 128, :], in_=ych[:])

            def ar_rows(q0, rows):
                nc.gpsimd.collective_compute(
                    "AllReduce", mybir.AluOpType.add, replica_groups=groups,
                    ins=[ypart[q0:q0 + rows, :].opt()],
                    outs=[yred[q0:q0 + rows, :].opt()])
                nc.sync.dma_start(out=y[q0:q0 + rows, :],
                                  in_=yred[q0:q0 + rows, :])

            # ---- schedule (emission order == per-engine queue order) ----
            for t in range(OT):
                qk_unit(0, "q", t)
            for t in range(OT):
                qk_unit(0, "k", t)
            for st in range(8):
                v_unit(st)
            for tp in range(OT):
                attn_tp(0, tp)
            attn_finish(0)
            for st in range(8, 16):
                v_unit(st)
            attn_tp(1, 0)
            qk_unit(1, "q", 0)
            qk_unit(1, "q", 1)
            attn_tp(1, 1)
            qk_unit(1, "q", 2)
            qk_unit(1, "q", 3)
            attn_tp(1, 2)
            qk_unit(1, "k", 0)
            qk_unit(1, "k", 1)
            attn_tp(1, 3)
            qk_unit(1, "k", 2)
            qk_unit(1, "k", 3)
            attn_finish(1)
            proj_piece(0, 0)
            proj_piece(0, 1)
            attn_tp(2, 0)
            proj_piece(0, 2)
            proj_piece(0, 3)
            ar_rows(0, 512)
            attn_tp(2, 1)
            attn_tp(2, 2)
            attn_tp(2, 3)
            attn_finish(2)
            attn_tp(3, 0)
            proj_piece(1, 0)
            proj_piece(1, 1)
            attn_tp(3, 1)
            proj_piece(1, 2)
            proj_piece(1, 3)
            ar_rows(512, 512)
            attn_tp(3, 2)
            proj_piece(2, 0)
            proj_piece(2, 1)
            attn_tp(3, 3)
            proj_piece(2, 2)
            proj_piece(2, 3)
            ar_rows(1024, 512)
            attn_finish(3)
            for r2 in range(4):
                proj_piece(3, r2)
                ar_rows(1536 + 128 * r2, 128)

    nc.compile()
    return nc


def _prep_inputs(x, Wq, Wk, Wv, Wo, cos_emb, sin_emb, token_positions):
    bf = ml_dtypes.bfloat16
    cos_g = np.asarray(cos_emb)[np.asarray(token_positions)]  # [S, DK]
    sin_g = np.asarray(sin_emb)[np.asarray(token_positions)]
    # [128, S]: partition p -> head-dim p % 64
    cosT = np.ascontiguousarray(np.tile(cos_g.T, (2, 1))).astype(bf)
    sinT = np.ascontiguousarray(np.tile(sin_g.T, (2, 1))).astype(bf)
    # rotate-half-interleaved as a matmul: rh = SWAP @ q (per 128-dim tile)
    swap = np.zeros((128, 128), np.float32)
    for j in range(64):
        swap[2 * j, 2 * j + 1] = -1.0
        swap[2 * j + 1, 2 * j] = 1.0
    swapT = np.ascontiguousarray(swap.T).astype(bf)
    # causal mask for the diagonal 128x128 block in S^T=[j,i] layout
    jj = np.arange(128)[:, None]
    ii = np.arange(128)[None, :]
    maskT = np.where(ii >= jj, 0.0, NEG).astype(np.float32)

    in_maps = []
    for c in range(NCORES):
        b, hh = c // 2, c % 2
        cols = slice(512 * hh, 512 * (hh + 1))
        in_maps.append({
            "xT": np.ascontiguousarray(np.asarray(x)[b].T).astype(bf),
            "wqT": np.ascontiguousarray(np.asarray(Wq)[cols, :].T).astype(bf),
            "wkT": np.ascontiguousarray(np.asarray(Wk)[cols, :].T).astype(bf),
            "wvT": np.ascontiguousarray(np.asarray(Wv)[cols, :].T).astype(bf),
            "woT": np.ascontiguousarray(np.asarray(Wo)[:, cols].T).astype(bf),
            "cosT": cosT, "sinT": sinT, "swapT": swapT, "maskT": maskT,
        })
    return in_maps


def kernel(x, Wq, Wk, Wv, Wo, cos_emb, sin_emb, token_positions, **run_kwargs):
    if "nc" not in _compiled:
        _compiled["nc"] = _build_nc()
    nc = _compiled["nc"]
    in_maps = _prep_inputs(x, Wq, Wk, Wv, Wo, cos_emb, sin_emb, token_positions)
    res = run_bass_kernel_spmd(nc, in_maps, list(range(NCORES)), **run_kwargs)
    out = np.stack([res.results[2 * b]["y"] for b in range(B)]).astype(np.float32)
    if run_kwargs:
        kernel.last_result = res
    return out


# revision 7
# speedup vs baseline: 1.1812x; 1.0511x over previous
"""Causal multi-head self-attention with RoPE on 8 NeuronCores.

Sharding: batch (4) x head-halves (2) -> 8 cores. Core c handles batch c//2,
heads [8*(c%2), 8*(c%2)+8). Pipelined schedule: QKV projection, attention
i-chunks and output projection are interleaved at tile granularity so the
Scalar engine (softmax exp) and PE (matmuls) overlap across the whole kernel.
The pair of cores sharing a batch exchanges normalized attention outputs with
a per-chunk AllGather; each core then projects its own half of the output
columns with the full 1024-dim contraction (no AllReduce, half the traffic).
"""

import numpy as np
import ml_dtypes

import concourse.bacc as bacc
import concourse.bass as bass
import concourse.mybir as mybir
from concourse.tile import TileContext
from concourse.bass_utils import run_bass_kernel_spmd

B, S, D, H = 4, 2048, 1024, 16
HL = 8          # heads per core
DK = 64         # head dim
NCORES = 8
DT = D // 128   # 8 d-tiles (contraction tiles)
OT = HL * DK // 128   # 4 o-tiles for Q^T/K^T ([128, S] each, 2 heads per tile)
ST = S // 128   # 16 s-tiles
VW = DK + 1     # V columns per head incl. ones column

BF16 = mybir.dt.bfloat16
F32 = mybir.dt.float32
NEG = -1.0e9
EXP = mybir.ActivationFunctionType.Exp

_compiled = {}


def _build_nc():
    nc = bacc.Bacc("TRN2", target_bir_lowering=False, debug=False,
                   num_devices=NCORES)

    xT = nc.dram_tensor("xT", [D, S], BF16, kind="ExternalInput")
    wqT = nc.dram_tensor("wqT", [D, HL * DK], BF16, kind="ExternalInput")
    wkT = nc.dram_tensor("wkT", [D, HL * DK], BF16, kind="ExternalInput")
    wvT = nc.dram_tensor("wvT", [D, HL * DK], BF16, kind="ExternalInput")
    woT = nc.dram_tensor("woT", [D, D // 2], BF16, kind="ExternalInput")
    cosT = nc.dram_tensor("cosT", [128, S], BF16, kind="ExternalInput")
    sinT = nc.dram_tensor("sinT", [128, S], BF16, kind="ExternalInput")
    swapT = nc.dram_tensor("swapT", [128, 128], BF16, kind="ExternalInput")
    maskT = nc.dram_tensor("maskT", [128, 128], F32, kind="ExternalInput")
    y = nc.dram_tensor("y", [S, D // 2], BF16, kind="ExternalOutput")

    groups = [[0, 1], [2, 3], [4, 5], [6, 7]]

    with TileContext(nc) as tc:
        with (
            tc.tile_pool(name="big", bufs=1) as big,
            tc.tile_pool(name="rawp", bufs=2) as rawp,
            tc.tile_pool(name="t1p", bufs=2) as t1p,
            tc.tile_pool(name="t2p", bufs=2) as t2p,
            tc.tile_pool(name="ptp", bufs=8) as ptp,
            tc.tile_pool(name="osbp", bufs=10) as osbp,
            tc.tile_pool(name="otp", bufs=8) as otp,
            tc.tile_pool(name="oap", bufs=8) as oap,
            tc.tile_pool(name="ychp", bufs=3) as ychp,
            tc.tile_pool(name="nrmp", bufs=2) as nrmp,
            tc.tile_pool(name="stgp", bufs=3) as stgp,
            tc.tile_pool(name="repp", bufs=3) as repp,
            tc.tile_pool(name="ps_b", bufs=3, space="PSUM") as ps_b,
            tc.tile_pool(name="ps_o", bufs=2, space="PSUM") as ps_o,
            tc.tile_pool(name="dram", bufs=1, space="DRAM") as dram,
        ):
            # ---- input DMAs, split across queues so QKV can start early ----
            w_sb = {"q": [None] * DT, "k": [None] * DT, "v": [None] * DT}
            xT_sb = [None] * DT
            for k in range(DT):
                e1, e2 = (nc.sync, nc.scalar) if k % 2 == 0 else (nc.scalar, nc.sync)
                xt = big.tile([128, S], BF16, tag=f"xT{k}", name=f"xT{k}")
                e1.dma_start(out=xt[:], in_=xT[128 * k:128 * (k + 1), :])
                xT_sb[k] = xt
                wt = big.tile([128, HL * DK], BF16, tag=f"wq{k}", name=f"wq{k}")
                e2.dma_start(out=wt[:], in_=wqT[128 * k:128 * (k + 1), :])
                w_sb["q"][k] = wt
            cos_sb = big.tile([128, S], BF16, tag="cos")
            nc.scalar.dma_start(out=cos_sb[:], in_=cosT[:])
            sin_sb = big.tile([128, S], BF16, tag="sin")
            nc.scalar.dma_start(out=sin_sb[:], in_=sinT[:])
            swap_sb = big.tile([128, 128], BF16, tag="swap")
            nc.scalar.dma_start(out=swap_sb[:], in_=swapT[:])
            for k in range(DT):
                e1 = nc.sync if k % 2 == 0 else nc.scalar
                wt = big.tile([128, HL * DK], BF16, tag=f"wk{k}", name=f"wk{k}")
                e1.dma_start(out=wt[:], in_=wkT[128 * k:128 * (k + 1), :])
                w_sb["k"][k] = wt
            mask_sb = big.tile([128, 128], F32, tag="mask")
            nc.scalar.dma_start(out=mask_sb[:], in_=maskT[:])
            for k in range(DT):
                e1 = nc.sync if k % 2 == 0 else nc.scalar
                wt = big.tile([128, HL * DK], BF16, tag=f"wv{k}", name=f"wv{k}")
                e1.dma_start(out=wt[:], in_=wvT[128 * k:128 * (k + 1), :])
                w_sb["v"][k] = wt
            woT_sb = []
            for k in range(DT):
                e1 = nc.sync if k % 2 == 0 else nc.scalar
                wt = big.tile([128, D // 2], BF16, tag=f"wo{k}", name=f"wo{k}")
                e1.dma_start(out=wt[:], in_=woT[128 * k:128 * (k + 1), :])
                woT_sb.append(wt)

            vaug = []
            for st in range(ST):
                vt = big.tile([128, HL * VW], BF16, tag=f"va{st}", name=f"va{st}")
                ones = vt[:].rearrange("p (h d) -> p h d", d=VW)[:, :, DK:VW]
                nc.gpsimd.memset(ones, 1.0)
                vaug.append(vt)
            qrot = [big.tile([128, S], BF16, tag=f"qr{t}", name=f"qr{t}")
                    for t in range(OT)]
            krot = [big.tile([128, S], BF16, tag=f"kr{t}", name=f"kr{t}")
                    for t in range(OT)]

            oT = {}        # m -> [OT tiles [128,512]] my normalized o chunk
            oPeer = {}     # m -> [OT tiles [128,512]] peer half via AllGather
            osb_map = {}   # (m, h) -> osb tile
            sums_map = {}  # m -> sums8 tile
            # chunk-major o exchange buffers: chunk m of og_in is rows
            # [512m, 512m+512) = my 512 head-dims, cols = 512 i's of chunk m
            og_in = dram.tile([4 * 512, 512], BF16)
            og_out = dram.tile([4 * 1024, 512], BF16)  # gathered pair per chunk

            def qk_unit(h, qk, t):
                c0 = 1024 * h
                ps = ps_b.tile([128, 1024], F32, tag="psb", name="ps")
                for k in range(DT):
                    for cc in range(2):
                        nc.tensor.matmul(
                            ps[:, 512 * cc:512 * (cc + 1)],
                            lhsT=w_sb[qk][k][:, 128 * t:128 * (t + 1)],
                            rhs=xT_sb[k][:, c0 + 512 * cc:c0 + 512 * (cc + 1)],
                            start=(k == 0), stop=(k == DT - 1))
                raw = rawp.tile([128, 1024], BF16, tag="raw")
                nc.scalar.copy(out=raw[:], in_=ps[:])
                t1 = t1p.tile([128, 1024], BF16, tag="t1")
                nc.vector.tensor_mul(t1[:], raw[:], cos_sb[:, c0:c0 + 1024])
                ps2 = ps_b.tile([128, 1024], F32, tag="psb", name="ps2")
                for cc in range(2):
                    nc.tensor.matmul(
                        ps2[:, 512 * cc:512 * (cc + 1)], lhsT=swap_sb[:],
                        rhs=raw[:, 512 * cc:512 * (cc + 1)],
                        start=True, stop=True)
                t2 = t2p.tile([128, 1024], BF16, tag="t2")
                nc.vector.tensor_mul(t2[:], ps2[:], sin_sb[:, c0:c0 + 1024])
                dst = qrot[t] if qk == "q" else krot[t]
                nc.vector.tensor_add(dst[:, c0:c0 + 1024], t1[:], t2[:])

            def v_unit(st):
                ps = ps_b.tile([128, 1024], F32, tag="psb", name="psv")[:, 0:512]
                for k in range(DT):
                    nc.tensor.matmul(
                        ps[:], lhsT=xT_sb[k][:, 128 * st:128 * (st + 1)],
                        rhs=w_sb["v"][k][:], start=(k == 0), stop=(k == DT - 1))
                dst = vaug[st][:].rearrange("p (h d) -> p h d", d=VW)[:, :, 0:DK]
                src = ps[:].rearrange("p (h d) -> p h d", d=DK)
                nc.vector.tensor_copy(dst, src)

            def attn_tp(m, tp):
                i0 = 512 * m
                njb = 4 * m + 4
                diags = list(range(4 * m, njb))
                fulls = list(range(0, 4 * m))
                order = diags + fulls
                ogrps = [order[i:i + 8] for i in range(0, len(order), 8)]
                o_pse = ps_o.tile([VW, 512], F32, tag="pso", name="opse")
                o_pso = ps_o.tile([VW, 512], F32, tag="pso", name="opso")
                started = False
                ndone = 0
                for grp in ogrps:
                    pts = {}
                    for jb in grp:
                        j0 = 128 * jb
                        dlt = max(0, j0 - i0)
                        s_ps = ps_b.tile([128, 1024], F32, tag="psb", name="sps")
                        for half, po in ((0, 0), (1, DK)):
                            nc.tensor.matmul(
                                s_ps[:, 512 * half + dlt:512 * (half + 1)],
                                lhsT=krot[tp][po:po + DK, j0:j0 + 128],
                                rhs=qrot[tp][po:po + DK, i0 + dlt:i0 + 512],
                                start=True, stop=True)
                        if j0 >= i0:
                            s3 = s_ps[:].rearrange("p (b f) -> p b f", b=2)
                            nc.vector.tensor_add(
                                s3[:, :, dlt:dlt + 128],
                                s3[:, :, dlt:dlt + 128],
                                mask_sb[:].rearrange("p (b f) -> p b f", b=1)
                                .broadcast_to([128, 2, 128]))
                        pT = ptp.tile([128, 1024], BF16, tag="pT")
                        nc.scalar.activation(
                            pT[:].rearrange("p (b f) -> p b f", b=2)[:, :, dlt:512],
                            s_ps[:].rearrange("p (b f) -> p b f", b=2)[:, :, dlt:512],
                            EXP, scale=0.125)
                        pts[jb] = pT
                    ndone += len(grp)
                    for jb in sorted(grp):
                        dlt = max(0, 128 * jb - i0)
                        last = (ndone == njb and jb == max(grp))
                        for half, o_ps in ((0, o_pse), (1, o_pso)):
                            nc.tensor.matmul(
                                o_ps[:, dlt:512],
                                lhsT=vaug[jb][:, VW * (2 * tp + half):
                                              VW * (2 * tp + half + 1)],
                                rhs=pts[jb][:, 512 * half + dlt:512 * (half + 1)],
                                start=(not started), stop=last,
                            )
                        started = True
                osb_e = osbp.tile([VW, 512], BF16, tag="osb", name="osbe")
                nc.vector.tensor_copy(osb_e[:], o_pse[:])
                osb_o = osbp.tile([VW, 512], BF16, tag="osb", name="osbo")
                nc.vector.tensor_copy(osb_o[:], o_pso[:])
                if m not in sums_map:
                    sums_map[m] = nrmp.tile([8, 512], BF16, tag="sums8",
                                            name=f"sums8_{m}")
                sums8 = sums_map[m]
                nc.sync.dma_start(out=sums8[2 * tp:2 * tp + 1, :],
                                  in_=osb_e[DK:VW, :])
                nc.sync.dma_start(out=sums8[2 * tp + 1:2 * tp + 2, :],
                                  in_=osb_o[DK:VW, :])
                osb_map[(m, 2 * tp)] = osb_e
                osb_map[(m, 2 * tp + 1)] = osb_o

            def attn_finish(m):
                i0 = 512 * m
                sums8 = sums_map[m]
                rec8 = nrmp.tile([8, 512], BF16, tag="rec8", name=f"rec8_{m}")
                with nc.allow_low_precision(reason="bf16 softmax denom ok"):
                    nc.vector.reciprocal(rec8[:], sums8[:])
                oT4 = [otp.tile([128, 512], BF16, tag="oT", name=f"oT{m}_{t}")
                       for t in range(OT)]
                for h in range(HL):
                    tp2, po = h // 2, DK * (h % 2)
                    stage = stgp.tile([1, 512], BF16, tag="stage")
                    nc.sync.dma_start(out=stage[:], in_=rec8[h:h + 1, :])
                    rep = repp.tile([64, 512], BF16, tag="rep")
                    nc.gpsimd.partition_broadcast(rep[:], stage[:])
                    nc.vector.tensor_mul(oT4[tp2][po:po + DK, :],
                                         osb_map[(m, h)][0:DK, :], rep[:])
                oT[m] = oT4
                # stage this chunk's o to DRAM for the pair AllGather
                for t in range(OT):
                    e1 = nc.sync if t % 2 == 0 else nc.scalar
                    e1.dma_start(out=og_in[512 * m + 128 * t:
                                           512 * m + 128 * t + 128, :],
                                 in_=oT4[t][:])

            def o_gather(m):
                nc.gpsimd.collective_compute(
                    "AllGather", mybir.AluOpType.bypass, replica_groups=groups,
                    ins=[og_in[512 * m:512 * m + 512, :].opt()],
                    outs=[og_out[1024 * m:1024 * m + 1024, :].opt()])

            # SPMD program is shared across cores, so the projection reads all
            # 8 gathered k-tiles from og_out in global head order (rank order
            # == head order: even core = heads 0-7 = rows 0-511).
            def o_fetch_all(m):
                o8 = [oap.tile([128, 512], BF16, tag="oa", name=f"oa{m}_{t}")
                      for t in range(DT)]
                for t in range(DT):
                    e1 = nc.sync if t % 2 == 0 else nc.scalar
                    e1.dma_start(
                        out=o8[t][:],
                        in_=og_out[1024 * m + 128 * t:1024 * m + 128 * t + 128,
                                   :])
                oPeer[m] = o8

            def proj_piece2(m, r2):
                r0 = 512 * m + 128 * r2
                ych = ychp.tile([128, 512], BF16, tag="ych")
                yp = ps_b.tile([128, 1024], F32, tag="psb", name="yp")[:, 0:512]
                for kt in range(DT):
                    nc.tensor.matmul(
                        yp[:],
                        lhsT=oPeer[m][kt][:, 128 * r2:128 * (r2 + 1)],
                        rhs=woT_sb[kt][:],
                        start=(kt == 0), stop=(kt == DT - 1))
                nc.vector.tensor_copy(ych[:], yp[:])
                nc.sync.dma_start(out=y[r0:r0 + 128, :], in_=ych[:])

            qk_unit(0, "q", 0)
            qk_unit(0, "k", 0)
            for st in range(4):
                v_unit(st)
            attn_tp(0, 0)
            qk_unit(0, "q", 1)
            qk_unit(0, "k", 1)
            attn_tp(0, 1)
            qk_unit(0, "q", 2)
            qk_unit(0, "k", 2)
            attn_tp(0, 2)
            qk_unit(0, "q", 3)
            qk_unit(0, "k", 3)
            attn_tp(0, 3)
            attn_finish(0)
            o_gather(0)
            for st in range(4, 8):
                v_unit(st)
            attn_tp(1, 0)
            for st in range(8, 12):
                v_unit(st)
            attn_tp(1, 1)
            for st in range(12, 16):
                v_unit(st)
            attn_tp(1, 2)
            qk_unit(1, "q", 0)
            qk_unit(1, "k", 0)
            attn_tp(1, 3)
            attn_finish(1)
            o_gather(1)
            o_fetch_all(0)
            qk_unit(1, "q", 1)
            qk_unit(1, "k", 1)
            proj_piece2(0, 0)
            proj_piece2(0, 1)
            qk_unit(1, "q", 2)
            qk_unit(1, "k", 2)
            proj_piece2(0, 2)
            proj_piece2(0, 3)
            qk_unit(1, "q", 3)
            qk_unit(1, "k", 3)
            attn_tp(2, 0)
            o_fetch_all(1)
            attn_tp(2, 1)
            proj_piece2(1, 0)
            proj_piece2(1, 1)
            attn_tp(2, 2)
            proj_piece2(1, 2)
            proj_piece2(1, 3)
            attn_tp(2, 3)
            attn_finish(2)
            o_gather(2)
            attn_tp(3, 0)
            o_fetch_all(2)
            attn_tp(3, 1)
            proj_piece2(2, 0)
            proj_piece2(2, 1)
            attn_tp(3, 2)
            proj_piece2(2, 2)
            proj_piece2(2, 3)
            attn_tp(3, 3)
            attn_finish(3)
            o_gather(3)
            o_fetch_all(3)
            for r2 in range(4):
                proj_piece2(3, r2)

    nc.compile()
    return nc


def _prep_inputs(x, Wq, Wk, Wv, Wo, cos_emb, sin_emb, token_positions):
    bf = ml_dtypes.bfloat16
    cos_g = np.asarray(cos_emb)[np.asarray(token_positions)]  # [S, DK]
    sin_g = np.asarray(sin_emb)[np.asarray(token_positions)]
    # [128, S]: partition p -> head-dim p % 64
    cosT = np.ascontiguousarray(np.tile(cos_g.T, (2, 1))).astype(bf)
    sinT = np.ascontiguousarray(np.tile(sin_g.T, (2, 1))).astype(bf)
    # rotate-half-interleaved as a matmul: rh = SWAP @ q (per 128-dim tile)
    swap = np.zeros((128, 128), np.float32)
    for j in range(64):
        swap[2 * j, 2 * j + 1] = -1.0
        swap[2 * j + 1, 2 * j] = 1.0
    swapT = np.ascontiguousarray(swap.T).astype(bf)
    # causal mask for the diagonal 128x128 block in S^T=[j,i] layout
    jj = np.arange(128)[:, None]
    ii = np.arange(128)[None, :]
    maskT = np.where(ii >= jj, 0.0, NEG).astype(np.float32)

    in_maps = []
    for c in range(NCORES):
        b, hh = c // 2, c % 2
        cols = slice(512 * hh, 512 * (hh + 1))   # my heads' dims
        ocols = slice(512 * hh, 512 * (hh + 1))  # my output columns
        in_maps.append({
            "xT": np.ascontiguousarray(np.asarray(x)[b].T).astype(bf),
            "wqT": np.ascontiguousarray(np.asarray(Wq)[cols, :].T).astype(bf),
            "wkT": np.ascontiguousarray(np.asarray(Wk)[cols, :].T).astype(bf),
            "wvT": np.ascontiguousarray(np.asarray(Wv)[cols, :].T).astype(bf),
            "woT": np.ascontiguousarray(np.asarray(Wo)[ocols, :].T).astype(bf),
            "cosT": cosT, "sinT": sinT, "swapT": swapT, "maskT": maskT,
        })
    return in_maps


def kernel(x, Wq, Wk, Wv, Wo, cos_emb, sin_emb, token_positions, **run_kwargs):
    if "nc" not in _compiled:
        _compiled["nc"] = _build_nc()
    nc = _compiled["nc"]
    in_maps = _prep_inputs(x, Wq, Wk, Wv, Wo, cos_emb, sin_emb, token_positions)
    res = run_bass_kernel_spmd(nc, in_maps, list(range(NCORES)), **run_kwargs)
    out = np.stack([
        np.concatenate([res.results[2 * b]["y"], res.results[2 * b + 1]["y"]],
                       axis=1)
        for b in range(B)
    ]).astype(np.float32)
    if run_kwargs:
        kernel.last_result = res
    return out


# revision 15
# speedup vs baseline: 1.2502x; 1.0584x over previous
"""Causal multi-head self-attention with RoPE on 8 NeuronCores.

Sharding: batch (4) x head-halves (2) -> 8 cores. Core c handles batch c//2,
heads [8*(c%2), 8*(c%2)+8). Pipelined schedule: QKV projection, attention
i-chunks and output projection are interleaved at tile granularity so the
Scalar engine (softmax exp) and PE (matmuls) overlap across the whole kernel.
The pair of cores sharing a batch exchanges normalized attention outputs with
a per-chunk AllGather; each core then projects its own half of the output
columns with the full 1024-dim contraction (no AllReduce, half the traffic).
"""

import numpy as np
import ml_dtypes

import concourse.bacc as bacc
import concourse.bass as bass
import concourse.mybir as mybir
from concourse.tile import TileContext
from concourse.bass_utils import run_bass_kernel_spmd

B, S, D, H = 4, 2048, 1024, 16
HL = 8          # heads per core
DK = 64         # head dim
NCORES = 8
DT = D // 128   # 8 d-tiles (contraction tiles)
OT = HL * DK // 128   # 4 o-tiles for Q^T/K^T ([128, S] each, 2 heads per tile)
ST = S // 128   # 16 s-tiles
VW = DK + 1     # V columns per head incl. ones column

BF16 = mybir.dt.bfloat16
F32 = mybir.dt.float32
NEG = -1.0e9
EXP = mybir.ActivationFunctionType.Exp

_compiled = {}


def _build_nc():
    nc = bacc.Bacc("TRN2", target_bir_lowering=False, debug=False,
                   num_devices=NCORES)

    xT = nc.dram_tensor("xT", [D, S], BF16, kind="ExternalInput")
    wqT = nc.dram_tensor("wqT", [D, HL * DK], BF16, kind="ExternalInput")
    wkT = nc.dram_tensor("wkT", [D, HL * DK], BF16, kind="ExternalInput")
    wvT = nc.dram_tensor("wvT", [D, HL * DK], BF16, kind="ExternalInput")
    woT = nc.dram_tensor("woT", [D, D // 2], BF16, kind="ExternalInput")
    cosT = nc.dram_tensor("cosT", [128, S], BF16, kind="ExternalInput")
    sinT = nc.dram_tensor("sinT", [128, S], BF16, kind="ExternalInput")
    swapT = nc.dram_tensor("swapT", [128, 128], BF16, kind="ExternalInput")
    maskT = nc.dram_tensor("maskT", [128, 128], BF16, kind="ExternalInput")
    y = nc.dram_tensor("y", [S, D // 2], BF16, kind="ExternalOutput")

    groups = [[0, 1], [2, 3], [4, 5], [6, 7]]

    with TileContext(nc) as tc:
        with (
            tc.tile_pool(name="big", bufs=1) as big,
            tc.tile_pool(name="rawp", bufs=2) as rawp,
            tc.tile_pool(name="t1p", bufs=2) as t1p,
            tc.tile_pool(name="t2p", bufs=2) as t2p,
            tc.tile_pool(name="ptp", bufs=12) as ptp,
            tc.tile_pool(name="osbp", bufs=10) as osbp,
            tc.tile_pool(name="otp", bufs=8) as otp,
            tc.tile_pool(name="oap", bufs=8) as oap,
            tc.tile_pool(name="ychp", bufs=3) as ychp,
            tc.tile_pool(name="nrmp", bufs=2) as nrmp,
            tc.tile_pool(name="stgp", bufs=3) as stgp,
            tc.tile_pool(name="repp", bufs=3) as repp,
            tc.tile_pool(name="ps_b", bufs=2, space="PSUM") as ps_b,
            tc.tile_pool(name="ps_o", bufs=4, space="PSUM") as ps_o,
            tc.tile_pool(name="dram", bufs=1, space="DRAM") as dram,
        ):
            # ---- input DMAs, split across queues so QKV can start early ----
            w_sb = {"q": [None] * DT, "k": [None] * DT, "v": [None] * DT}
            xT_sb = [None] * DT
            for k in range(DT):
                e1, e2 = (nc.sync, nc.scalar) if k % 2 == 0 else (nc.scalar, nc.sync)
                xt = big.tile([128, S], BF16, tag=f"xT{k}", name=f"xT{k}")
                e1.dma_start(out=xt[:], in_=xT[128 * k:128 * (k + 1), :])
                xT_sb[k] = xt
                wt = big.tile([128, HL * DK], BF16, tag=f"wq{k}", name=f"wq{k}")
                e2.dma_start(out=wt[:], in_=wqT[128 * k:128 * (k + 1), :])
                w_sb["q"][k] = wt
            cos_sb = big.tile([128, S], BF16, tag="cos")
            nc.scalar.dma_start(out=cos_sb[:], in_=cosT[:])
            sin_sb = big.tile([128, S], BF16, tag="sin")
            nc.scalar.dma_start(out=sin_sb[:], in_=sinT[:])
            swap_sb = big.tile([128, 128], BF16, tag="swap")
            nc.scalar.dma_start(out=swap_sb[:], in_=swapT[:])
            for k in range(DT):
                e1 = nc.sync if k % 2 == 0 else nc.scalar
                wt = big.tile([128, HL * DK], BF16, tag=f"wk{k}", name=f"wk{k}")
                e1.dma_start(out=wt[:], in_=wkT[128 * k:128 * (k + 1), :])
                w_sb["k"][k] = wt
            mask_sb = big.tile([128, 128], BF16, tag="mask")
            nc.scalar.dma_start(out=mask_sb[:], in_=maskT[:])
            for k in range(DT):
                e1 = nc.sync if k % 2 == 0 else nc.scalar
                wt = big.tile([128, HL * DK], BF16, tag=f"wv{k}", name=f"wv{k}")
                e1.dma_start(out=wt[:], in_=wvT[128 * k:128 * (k + 1), :])
                w_sb["v"][k] = wt
            woT_sb = []
            for k in range(DT):
                e1 = nc.sync if k % 2 == 0 else nc.scalar
                wt = big.tile([128, D // 2], BF16, tag=f"wo{k}", name=f"wo{k}")
                e1.dma_start(out=wt[:], in_=woT[128 * k:128 * (k + 1), :])
                woT_sb.append(wt)

            vaug = []
            for st in range(ST):
                vt = big.tile([128, HL * VW], BF16, tag=f"va{st}", name=f"va{st}")
                ones = vt[:].rearrange("p (h d) -> p h d", d=VW)[:, :, DK:VW]
                nc.gpsimd.memset(ones, 1.0)
                vaug.append(vt)
            qrot = [big.tile([128, S], BF16, tag=f"qr{t}", name=f"qr{t}")
                    for t in range(OT)]
            krot = [big.tile([128, S], BF16, tag=f"kr{t}", name=f"kr{t}")
                    for t in range(OT)]

            oT = {}        # m -> [OT tiles [128,512]] my normalized o chunk
            oPeer = {}     # m -> [OT tiles [128,512]] peer half via AllGather
            osb_map = {}   # (m, h) -> osb tile
            sums_map = {}  # m -> sums8 tile
            # chunk-major o exchange buffers: chunk m of og_in is rows
            # [512m, 512m+512) = my 512 head-dims, cols = 512 i's of chunk m
            og_in = dram.tile([4 * 512, 512], BF16)
            og_out = dram.tile([4 * 1024, 512], BF16)  # gathered pair per chunk

            qk_state = {}

            def qk_proj(h, qk, t):
                c0 = 1024 * h
                ps = ps_b.tile([128, 1024], F32, tag="psb", name="ps")
                for k in range(DT):
                    for cc in range(2):
                        nc.tensor.matmul(
                            ps[:, 512 * cc:512 * (cc + 1)],
                            lhsT=w_sb[qk][k][:, 128 * t:128 * (t + 1)],
                            rhs=xT_sb[k][:, c0 + 512 * cc:c0 + 512 * (cc + 1)],
                            start=(k == 0), stop=(k == DT - 1))
                raw = rawp.tile([128, 1024], BF16, tag="raw")
                nc.scalar.copy(out=raw[:], in_=ps[:])
                t1 = t1p.tile([128, 1024], BF16, tag="t1")
                nc.vector.tensor_mul(t1[:], raw[:], cos_sb[:, c0:c0 + 1024])
                qk_state[(h, qk, t)] = (raw, t1)

            def qk_swap(h, qk, t):
                c0 = 1024 * h
                raw, t1 = qk_state.pop((h, qk, t))
                ps2 = ps_b.tile([128, 1024], F32, tag="psb", name="ps2")
                for cc in range(2):
                    nc.tensor.matmul(
                        ps2[:, 512 * cc:512 * (cc + 1)], lhsT=swap_sb[:],
                        rhs=raw[:, 512 * cc:512 * (cc + 1)],
                        start=True, stop=True)
                t2 = t2p.tile([128, 1024], BF16, tag="t2")
                nc.vector.tensor_mul(t2[:], ps2[:], sin_sb[:, c0:c0 + 1024])
                dst = qrot[t] if qk == "q" else krot[t]
                nc.vector.tensor_add(dst[:, c0:c0 + 1024], t1[:], t2[:])

            def qk_unit(h, qk, t):
                qk_proj(h, qk, t)
                qk_swap(h, qk, t)

            def v_unit(st):
                ps = ps_b.tile([128, 1024], F32, tag="psb", name="psv")[:, 0:512]
                for k in range(DT):
                    nc.tensor.matmul(
                        ps[:], lhsT=xT_sb[k][:, 128 * st:128 * (st + 1)],
                        rhs=w_sb["v"][k][:], start=(k == 0), stop=(k == DT - 1))
                dst = vaug[st][:].rearrange("p (h d) -> p h d", d=VW)[:, :, 0:DK]
                src = ps[:].rearrange("p (h d) -> p h d", d=DK)
                nc.vector.tensor_copy(dst, src)

            def _attn_slot(m, tp, jb):
                # QK^T (row-tiled head pair) -> exp -> post-exp 0/1 mask
                i0 = 512 * m
                j0 = 128 * jb
                dlt = max(0, j0 - i0)
                s_ps = ps_b.tile([128, 1024], F32, tag="psb", name="sps")
                for half, po in ((0, 0), (1, DK)):
                    nc.tensor.matmul(
                        s_ps[:, 512 * half + dlt:512 * (half + 1)],
                        lhsT=krot[tp][po:po + DK, j0:j0 + 128],
                        rhs=qrot[tp][po:po + DK, i0 + dlt:i0 + 512],
                        start=True, stop=True)
                pT = ptp.tile([128, 1024], BF16, tag="pT")
                pT3 = pT[:].rearrange("p (b f) -> p b f", b=2)
                nc.scalar.activation(
                    pT3[:, :, dlt:512],
                    s_ps[:].rearrange("p (b f) -> p b f", b=2)[:, :, dlt:512],
                    EXP, scale=0.125)
                if j0 >= i0:
                    nc.vector.tensor_mul(
                        pT3[:, :, dlt:dlt + 128],
                        pT3[:, :, dlt:dlt + 128],
                        mask_sb[:].rearrange("p (b f) -> p b f", b=1)
                        .broadcast_to([128, 2, 128]))
                return pT

            def _attn_pv(m, tp, jb, pT, accs, start, stop):
                dlt = max(0, 128 * jb - 512 * m)
                for half in (0, 1):
                    nc.tensor.matmul(
                        accs[half][:, dlt:512],
                        lhsT=vaug[jb][:, VW * (2 * tp + half):
                                      VW * (2 * tp + half + 1)],
                        rhs=pT[:, 512 * half + dlt:512 * (half + 1)],
                        start=start, stop=stop,
                    )

            def _attn_drain(m, tp, accs):
                osb_e = osbp.tile([VW, 512], BF16, tag="osb", name="osbe")
                nc.vector.tensor_copy(osb_e[:], accs[0][:])
                osb_o = osbp.tile([VW, 512], BF16, tag="osb", name="osbo")
                nc.vector.tensor_copy(osb_o[:], accs[1][:])
                if m not in sums_map:
                    sums_map[m] = nrmp.tile([8, 512], BF16, tag="sums8",
                                            name=f"sums8_{m}")
                sums8 = sums_map[m]
                nc.sync.dma_start(out=sums8[2 * tp:2 * tp + 1, :],
                                  in_=osb_e[DK:VW, :])
                nc.sync.dma_start(out=sums8[2 * tp + 1:2 * tp + 2, :],
                                  in_=osb_o[DK:VW, :])
                osb_map[(m, 2 * tp)] = osb_e
                osb_map[(m, 2 * tp + 1)] = osb_o

            def attn_pair(m, tps, fillers=()):
                # two head-pair tiles interleaved slot-by-slot so the PE queue
                # always has ready work; PV in groups of 4 j-blocks per tp
                i0 = 512 * m
                njb = 4 * m + 4
                order = list(range(4 * m, njb)) + list(range(0, 4 * m))
                ogrps = [order[i:i + 4] for i in range(0, len(order), 4)]
                accs = {tp: (ps_o.tile([VW, 512], F32, tag="pso",
                                       name=f"oe{m}_{tp}"),
                             ps_o.tile([VW, 512], F32, tag="pso",
                                       name=f"oo{m}_{tp}")) for tp in tps}
                started = {tp: False for tp in tps}
                ndone = 0
                fill = list(fillers)
                for grp in ogrps:
                    pts = {}
                    for jb in grp:
                        for tp in tps:
                            pts[(tp, jb)] = _attn_slot(m, tp, jb)
                    ndone += len(grp)
                    for tp in tps:
                        for jb in sorted(grp):
                            last = (ndone == njb and jb == max(grp))
                            _attn_pv(m, tp, jb, pts[(tp, jb)], accs[tp],
                                     start=(not started[tp]), stop=last)
                            started[tp] = True
                    if fill:
                        fill.pop(0)()
                for tp in tps:
                    _attn_drain(m, tp, accs[tp])

            def attn_finish(m):
                i0 = 512 * m
                sums8 = sums_map[m]
                rec8 = nrmp.tile([8, 512], BF16, tag="rec8", name=f"rec8_{m}")
                with nc.allow_low_precision(reason="bf16 softmax denom ok"):
                    nc.vector.reciprocal(rec8[:], sums8[:])
                oT4 = [otp.tile([128, 512], BF16, tag="oT", name=f"oT{m}_{t}")
                       for t in range(OT)]
                for h in range(HL):
                    tp2, po = h // 2, DK * (h % 2)
                    stage = stgp.tile([1, 512], BF16, tag="stage")
                    nc.sync.dma_start(out=stage[:], in_=rec8[h:h + 1, :])
                    rep = repp.tile([64, 512], BF16, tag="rep")
                    nc.gpsimd.partition_broadcast(rep[:], stage[:])
                    nc.vector.tensor_mul(oT4[tp2][po:po + DK, :],
                                         osb_map[(m, h)][0:DK, :], rep[:])
                oT[m] = oT4
                # stage this chunk's o to DRAM for the pair AllGather
                for t in range(OT):
                    e1 = nc.sync if t % 2 == 0 else nc.scalar
                    e1.dma_start(out=og_in[512 * m + 128 * t:
                                           512 * m + 128 * t + 128, :],
                                 in_=oT4[t][:])

            def o_gather(m):
                nc.gpsimd.collective_compute(
                    "AllGather", mybir.AluOpType.bypass, replica_groups=groups,
                    ins=[og_in[512 * m:512 * m + 512, :].opt()],
                    outs=[og_out[1024 * m:1024 * m + 1024, :].opt()])

            # SPMD program is shared across cores, so the projection reads all
            # 8 gathered k-tiles from og_out in global head order (rank order
            # == head order: even core = heads 0-7 = rows 0-511).
            def o_fetch_all(m):
                o8 = [oap.tile([128, 512], BF16, tag="oa", name=f"oa{m}_{t}")
                      for t in range(DT)]
                for t in range(DT):
                    e1 = nc.sync if t % 2 == 0 else nc.scalar
                    e1.dma_start(
                        out=o8[t][:],
                        in_=og_out[1024 * m + 128 * t:1024 * m + 128 * t + 128,
                                   :])
                oPeer[m] = o8

            def proj_piece2(m, r2):
                r0 = 512 * m + 128 * r2
                ych = ychp.tile([128, 512], BF16, tag="ych")
                yp = ps_b.tile([128, 1024], F32, tag="psb", name="yp")[:, 0:512]
                for kt in range(DT):
                    nc.tensor.matmul(
                        yp[:],
                        lhsT=oPeer[m][kt][:, 128 * r2:128 * (r2 + 1)],
                        rhs=woT_sb[kt][:],
                        start=(kt == 0), stop=(kt == DT - 1))
                nc.vector.tensor_copy(ych[:], yp[:])
                nc.sync.dma_start(out=y[r0:r0 + 128, :], in_=ych[:])

            def qk_group(units):
                # software-pipeline proj/swap phases across a group of units
                # so the PE never waits on a single PSUM buf's evacuation
                qk_proj(*units[0])
                for i in range(1, len(units)):
                    qk_proj(*units[i])
                    qk_swap(*units[i - 1])
                qk_swap(*units[-1])

            qk_group([(0, "q", 0), (0, "k", 0), (0, "q", 1), (0, "k", 1)])
            for st in range(4):
                v_unit(st)
            attn_pair(0, (0, 1))
            qk_group([(0, "q", 2), (0, "k", 2), (0, "q", 3), (0, "k", 3)])
            attn_pair(0, (2, 3))
            attn_finish(0)
            o_gather(0)
            for st in range(4, 8):
                v_unit(st)
            attn_pair(1, (0, 1))
            for st in range(8, 12):
                v_unit(st)
            attn_pair(1, (2, 3))
            attn_finish(1)
            o_gather(1)
            for st in range(12, 16):
                v_unit(st)
            qk_group([(1, "q", 0), (1, "k", 0), (1, "q", 1), (1, "k", 1)])
            o_fetch_all(0)
            proj_piece2(0, 0)
            proj_piece2(0, 1)
            proj_piece2(0, 2)
            proj_piece2(0, 3)
            qk_group([(1, "q", 2), (1, "k", 2), (1, "q", 3), (1, "k", 3)])
            attn_pair(2, (0, 1))
            o_fetch_all(1)
            proj_piece2(1, 0)
            proj_piece2(1, 1)
            proj_piece2(1, 2)
            proj_piece2(1, 3)
            attn_pair(2, (2, 3))
            attn_finish(2)
            o_gather(2)
            attn_pair(3, (0, 1))
            o_fetch_all(2)
            proj_piece2(2, 0)
            proj_piece2(2, 1)
            proj_piece2(2, 2)
            proj_piece2(2, 3)
            attn_pair(3, (2, 3))
            attn_finish(3)
            o_gather(3)
            o_fetch_all(3)
            for r2 in range(4):
                proj_piece2(3, r2)

    nc.compile()
    return nc


def _prep_inputs(x, Wq, Wk, Wv, Wo, cos_emb, sin_emb, token_positions):
    bf = ml_dtypes.bfloat16
    cos_g = np.asarray(cos_emb)[np.asarray(token_positions)]  # [S, DK]
    sin_g = np.asarray(sin_emb)[np.asarray(token_positions)]
    # [128, S]: partition p -> head-dim p % 64
    cosT = np.ascontiguousarray(np.tile(cos_g.T, (2, 1))).astype(bf)
    sinT = np.ascontiguousarray(np.tile(sin_g.T, (2, 1))).astype(bf)
    # rotate-half-interleaved as a matmul: rh = SWAP @ q (per 128-dim tile)
    swap = np.zeros((128, 128), np.float32)
    for j in range(64):
        swap[2 * j, 2 * j + 1] = -1.0
        swap[2 * j + 1, 2 * j] = 1.0
    swapT = np.ascontiguousarray(swap.T).astype(bf)
    # causal 0/1 mask for the diagonal 128x128 block in S^T=[j,i] layout,
    # applied multiplicatively AFTER the exp
    jj = np.arange(128)[:, None]
    ii = np.arange(128)[None, :]
    maskT = np.where(ii >= jj, 1.0, 0.0).astype(bf)

    in_maps = []
    for c in range(NCORES):
        b, hh = c // 2, c % 2
        cols = slice(512 * hh, 512 * (hh + 1))   # my heads' dims
        ocols = slice(512 * hh, 512 * (hh + 1))  # my output columns
        in_maps.append({
            "xT": np.ascontiguousarray(np.asarray(x)[b].T).astype(bf),
            "wqT": np.ascontiguousarray(np.asarray(Wq)[cols, :].T).astype(bf),
            "wkT": np.ascontiguousarray(np.asarray(Wk)[cols, :].T).astype(bf),
            "wvT": np.ascontiguousarray(np.asarray(Wv)[cols, :].T).astype(bf),
            "woT": np.ascontiguousarray(np.asarray(Wo)[ocols, :].T).astype(bf),
            "cosT": cosT, "sinT": sinT, "swapT": swapT, "maskT": maskT,
        })
    return in_maps


def kernel(x, Wq, Wk, Wv, Wo, cos_emb, sin_emb, token_positions, **run_kwargs):
    if "nc" not in _compiled:
        _compiled["nc"] = _build_nc()
    nc = _compiled["nc"]
    in_maps = _prep_inputs(x, Wq, Wk, Wv, Wo, cos_emb, sin_emb, token_positions)
    res = run_bass_kernel_spmd(nc, in_maps, list(range(NCORES)), **run_kwargs)
    out = np.stack([
        np.concatenate([res.results[2 * b]["y"], res.results[2 * b + 1]["y"]],
                       axis=1)
        for b in range(B)
    ]).astype(np.float32)
    if run_kwargs:
        kernel.last_result = res
    return out
